# revision 1
# baseline (speedup 1.0000x reference)
"""BiGRU (N=64, T=512, D=512, H=512) on 8 TRN2 NeuronCores.

Sharding: data-parallel over batch (8 per core); each core runs both
directions as two interleaved GRU chains (chain0 = fwd, chain1 = bwd on
host-time-flipped x). Weights replicated (bf16), full T scan on-core.

Per chain step (batch 8):
  - gates psum [128,512] = [z_pre | r_pre | h_g | x_g]: 48 column-tiled
    matmuls (4 strips x 4 k-chunks x {W_h zrg(384), W_x zr(256), W_x g(128)}),
    stationary = h.T / x_t.T slices [128,8] bf16, moving = weight slices.
    The input projection x_t @ W_x is fused into the scan (never
    materialized in DRAM).
  - zr = sigmoid(ps[:,0:256]); g = tanh(r * ps[:,256:384] + ps[:,384:512])
  - h = g + z * (h - g)   (persistent fp32 [4 strips x 32 part, 128 units])
  - h transposed back to stationary layout with 4 col-tiled matmuls against
    a 0/1 selection matrix; fp32 copy staged to SBUF and DMA'd to the output.
"""

from contextlib import ExitStack

import numpy as np
import ml_dtypes

import concourse.bacc as bacc
import concourse.bass as bass
import concourse.tile as tile
import concourse.mybir as mybir
from concourse import bass_utils

F32 = mybir.dt.float32
BF16 = mybir.dt.bfloat16
AF = mybir.ActivationFunctionType
ALU = mybir.AluOpType

N_CORES = 8
N, T, D, H = 64, 512, 512, 512
U = 8  # time steps per DMA block / loop-body unroll


def build_gru(T_, U_, repeats=1, with_bias=False):
    assert T_ % U_ == 0
    nc = bacc.Bacc("TRN2", target_bir_lowering=False, debug=False,
                   num_devices=N_CORES)
    xs, wxs, whs, outs, bds = [], [], [], [], []
    for c in range(2):
        xs.append(nc.dram_tensor(f"x{c}", [T_ // U_, 128, U_, 4, 8], BF16,
                                 kind="ExternalInput").ap())
        wxs.append(nc.dram_tensor(f"wx{c}", [4, 128, 1536], BF16,
                                  kind="ExternalInput").ap())
        whs.append(nc.dram_tensor(f"wh{c}", [4, 128, 1536], BF16,
                                  kind="ExternalInput").ap())
        outs.append(nc.dram_tensor(f"out{c}", [T_, 128, 4, 8], F32,
                                   kind="ExternalOutput").ap())
        if with_bias:
            bds.append(nc.dram_tensor(f"b{c}", [1, 1536], BF16,
                                      kind="ExternalInput").ap())
    isel_d = nc.dram_tensor("isel", [128, 32], F32, kind="ExternalInput").ap()

    with tile.TileContext(nc) as tc, ExitStack() as ctx:
        cpool = ctx.enter_context(tc.tile_pool(name="const", bufs=1))
        xpools = [ctx.enter_context(tc.tile_pool(name=f"x{c}", bufs=2))
                  for c in range(2)]
        pspools = [ctx.enter_context(
            tc.tile_pool(name=f"ps{c}", bufs=2, space="PSUM"))
            for c in range(2)]
        ptpools = [ctx.enter_context(
            tc.tile_pool(name=f"pt{c}", bufs=2, space="PSUM"))
            for c in range(2)]
        epool = ctx.enter_context(tc.tile_pool(name="elem", bufs=3))

        isel = cpool.tile([128, 32], F32, tag="isel")
        nc.sync.dma_start(isel[:], isel_d[:])
        wx_sb, wh_sb, b_sb = [], [], []
        for c in range(2):
            wx_sb.append([cpool.tile([128, 1536], BF16, tag=f"wx{c}k{k}",
                                     name=f"wx{c}k{k}") for k in range(4)])
            wh_sb.append([cpool.tile([128, 1536], BF16, tag=f"wh{c}k{k}",
                                     name=f"wh{c}k{k}") for k in range(4)])
            for k in range(4):
                nc.sync.dma_start(wx_sb[c][k][:], wxs[c][k])
                nc.sync.dma_start(wh_sb[c][k][:], whs[c][k])
            if with_bias:
                bt = cpool.tile([1, 1536], BF16, tag=f"b{c}", name=f"b{c}")
                nc.sync.dma_start(bt[:], bds[c][:])
                b_sb.append(bt)
        if with_bias:
            ones = cpool.tile([1, 8], BF16, tag="ones")
            nc.vector.memset(ones[:], 1.0)

        h_state = [cpool.tile([128, 128], F32, tag=f"h{c}", name=f"h{c}")
                   for c in range(2)]
        hT_sb = [cpool.tile([128, 128], BF16, tag=f"hT{c}", name=f"hT{c}")
                 for c in range(2)]
        for c in range(2):
            nc.vector.memset(h_state[c][:], 0.0)
            nc.vector.memset(hT_sb[c][:], 0.0)

        def emit_step(c, x_tile, tl, t_dyn):
            ps = pspools[c].tile([128, 512], F32, tag=f"ps{c}", name="ps")
            for j in range(4):
                # One accumulation group per strip per step (psum start/stop
                # act on the whole 2KB bank per partition): first MM starts,
                # last MM stops, everything else accumulates.
                mms = []
                for k in range(4):
                    sh = hT_sb[c][:, 32 * k:32 * k + 8]
                    sx = x_tile[:, (tl * 4 + k) * 8:(tl * 4 + k) * 8 + 8]
                    mms.append((ps[32 * j:32 * j + 8, 0:384], sh,
                                wh_sb[c][k][:, 384 * j:384 * j + 384]))
                    mms.append((ps[32 * j:32 * j + 8, 0:256], sx,
                                wx_sb[c][k][:, 384 * j:384 * j + 256]))
                    mms.append((ps[32 * j:32 * j + 8, 384:512], sx,
                                wx_sb[c][k][:, 384 * j + 256:384 * j + 384]))
                if with_bias:
                    mms.append((ps[32 * j:32 * j + 8, 0:256], ones[:],
                                b_sb[c][:, 384 * j:384 * j + 256]))
                    mms.append((ps[32 * j:32 * j + 8, 384:512], ones[:],
                                b_sb[c][:, 384 * j + 256:384 * j + 384]))
                for idx, (o, lt, rh) in enumerate(mms):
                    nc.tensor.matmul(o, lhsT=lt, rhs=rh,
                                     start=(idx == 0),
                                     stop=(idx == len(mms) - 1),
                                     tile_position=(0, 32 * j))

            zr = epool.tile([128, 256], F32, tag=f"zr{c}", name="zr")
            nc.scalar.activation(zr[:], ps[:, 0:256], AF.Sigmoid)
            t1 = epool.tile([128, 128], F32, tag=f"t1{c}", name="t1")
            nc.vector.tensor_tensor(t1[:], zr[:, 128:256], ps[:, 256:384],
                                    ALU.mult)
            gp = epool.tile([128, 128], F32, tag=f"gp{c}", name="gp")
            nc.vector.tensor_tensor(gp[:], t1[:], ps[:, 384:512], ALU.add)
            g = epool.tile([128, 128], F32, tag=f"g{c}", name="g")
            nc.scalar.activation(g[:], gp[:], AF.Tanh)
            dtl = epool.tile([128, 128], F32, tag=f"d{c}", name="dtl")
            nc.vector.tensor_tensor(dtl[:], h_state[c][:], g[:], ALU.subtract)
            m = epool.tile([128, 128], F32, tag=f"m{c}", name="m")
            nc.vector.tensor_tensor(m[:], zr[:, 0:128], dtl[:], ALU.mult)
            nc.vector.tensor_tensor(h_state[c][:], m[:], g[:], ALU.add)

            pt = ptpools[c].tile([128, 32], F32, tag=f"pt{c}", name="pt")
            for mb in range(4):
                nc.tensor.matmul(
                    pt[32 * mb:32 * mb + 32, :],
                    lhsT=h_state[c][:, 32 * mb:32 * mb + 32],
                    rhs=isel[:], start=True, stop=True,
                    tile_position=(0, 32 * mb))
            hT_view = hT_sb[c][:].rearrange("p (k w) -> p k w", k=4)
            nc.vector.tensor_copy(
                hT_view[:, :, 0:8],
                pt[:].rearrange("p (s b) -> p s b", s=4))
            hTf = epool.tile([128, 32], F32, tag=f"hTf{c}", name="hTf")
            nc.scalar.copy(hTf[:], pt[:])
            dst = outs[c][bass.ds(t_dyn, 1)].rearrange(
                "o p s b -> (o p) s b")
            nc.sync.dma_start(dst, hTf[:].rearrange("p (s b) -> p s b", s=4))

        def time_block(i_dyn):
            x_tiles = []
            for c in range(2):
                xt = xpools[c].tile([128, U_ * 32], BF16, tag=f"xt{c}",
                                    name=f"xt{c}")
                src = xs[c][bass.ds(i_dyn, 1)].rearrange(
                    "o p u k n -> (o p) (u k n)")
                nc.sync.dma_start(xt[:], src)
                x_tiles.append(xt)
            for tl in range(U_):
                for c in range(2):
                    emit_step(c, x_tiles[c], tl, i_dyn * U_ + tl)

        n_blocks = T_ // U_
        if repeats == 1:
            with tc.For_i(0, n_blocks) as i:
                time_block(i)
        else:
            with tc.For_i(0, repeats) as rr:
                with tc.For_i(0, n_blocks) as i:
                    time_block(i)
    nc.compile()
    return nc


def arrange_w(w):
    """[512, 1536] -> [4, 128, 1536]: k-chunk, d', strip-major [z|r|g]."""
    w = np.asarray(w, np.float32).reshape(4, 128, 3, 4, 128)
    w = w.transpose(0, 1, 3, 2, 4).reshape(4, 128, 1536)
    return np.ascontiguousarray(w).astype(ml_dtypes.bfloat16)


def arrange_b(b):
    b = np.asarray(b, np.float32).reshape(3, 4, 128).transpose(1, 0, 2)
    return np.ascontiguousarray(b.reshape(1, 1536)).astype(ml_dtypes.bfloat16)


def arrange_x_all(x, U_):
    """[N, T, D] f32 -> [T//U, 128, U, 4, N] bf16 (slice batch last)."""
    n, t, _ = x.shape
    xt = np.transpose(x, (1, 2, 0)).reshape(t // U_, U_, 4, 128, n)
    return np.ascontiguousarray(xt.transpose(0, 3, 1, 2, 4)).astype(
        ml_dtypes.bfloat16)


def make_isel():
    isel = np.zeros((128, 32), np.float32)
    for s in range(4):
        for b in range(8):
            isel[32 * s + b, 8 * s + b] = 1.0
    return isel


def decode_out(o):
    """[T, 128, 4, 8] -> [8, T, 512] via h[b,t,128s+p] = o[t,p,s,b]."""
    t = o.shape[0]
    return np.ascontiguousarray(o.transpose(3, 0, 2, 1).reshape(8, t, 512))


_CACHE = {}


def _get_program(with_bias):
    key = ("prog", with_bias)
    if key not in _CACHE:
        _CACHE[key] = build_gru(T, U, repeats=1, with_bias=with_bias)
    return _CACHE[key]


def kernel(x, W_x_fwd, W_h_fwd, b_fwd, W_x_bwd, W_h_bwd, b_bwd):
    x = np.asarray(x, np.float32)
    W_x_fwd = np.asarray(W_x_fwd, np.float32)
    W_h_fwd = np.asarray(W_h_fwd, np.float32)
    W_x_bwd = np.asarray(W_x_bwd, np.float32)
    W_h_bwd = np.asarray(W_h_bwd, np.float32)
    b_fwd = np.asarray(b_fwd, np.float32)
    b_bwd = np.asarray(b_bwd, np.float32)
    assert x.shape == (N, T, D), x.shape

    with_bias = bool(np.any(b_fwd) or np.any(b_bwd))
    nc = _get_program(with_bias)

    x_fwd = arrange_x_all(x, U)                  # [T//U,128,U,4,64]
    x_bwd = arrange_x_all(x[:, ::-1], U)
    base = {
        "wx0": arrange_w(W_x_fwd), "wh0": arrange_w(W_h_fwd),
        "wx1": arrange_w(W_x_bwd), "wh1": arrange_w(W_h_bwd),
        "isel": make_isel(),
    }
    if with_bias:
        base["b0"] = arrange_b(b_fwd)
        base["b1"] = arrange_b(b_bwd)
    in_maps = []
    for c in range(N_CORES):
        m = dict(base)
        m["x0"] = np.ascontiguousarray(x_fwd[..., 8 * c:8 * c + 8])
        m["x1"] = np.ascontiguousarray(x_bwd[..., 8 * c:8 * c + 8])
        in_maps.append(m)

    res = bass_utils.run_bass_kernel_spmd(nc, in_maps,
                                          core_ids=list(range(N_CORES)))
    out = np.empty((N, T, 2 * H), np.float32)
    for c in range(N_CORES):
        sl = slice(8 * c, 8 * c + 8)
        out[sl, :, :H] = decode_out(res.results[c]["out0"])
        out[sl, :, H:] = decode_out(res.results[c]["out1"])[:, ::-1]
    return out

